# revision 12
# baseline (speedup 1.0000x reference)
"""Trainium2 Bass kernel: 3x3 conv2d (stride 1, pad 1), NCHW.

x (32, 64, 112, 112) f32, weight (1, 128, 64, 3, 3) f32 -> out (32, 128, 112, 112) f32.

Strategy: data-parallel over batch across 8 cores (4 images/core).
Per core, conv is computed as 9 PSUM-accumulating matmuls (one per kernel
tap): x is host-padded to (114, 114) so each tap's shifted input window is a
constant free-dim offset into the flat [in_c=64, 114*114] SBUF image. Output
is produced in padded row-major (112 x 114) layout and sliced on the host.
"""

import numpy as np

import concourse.bacc as bacc
import concourse.tile as tile
from concourse import mybir
from concourse.bass_utils import run_bass_kernel_spmd

# Problem constants (hardcoded per harness contract).
B, C, H, W = 32, 64, 112, 112
OC, KH, KW = 128, 3, 3
NCORES = 8
BPC = B // NCORES          # images per core
HP, WP = H + 2, W + 2      # host-padded input height/width (114)
XFLAT = HP * WP            # 12996 flat padded-input elements per channel
OFLAT = H * WP             # 12768 flat padded-output elements per channel
BLK = 512                  # matmul free-dim block (= 1 PSUM bank of fp32)
NBLK = (OFLAT + BLK - 1) // BLK  # 25 blocks (24 full + 1 of 480)

# matmul dtype: float32r streams fp32 through the PE at 1 cycle/row for
# free-dim >= 256 (vs 4 cycles/row for plain float32).
MM_DTYPE = mybir.dt.float32r

_cache = {}


def _build(repeat=1):
    """Build + compile the per-core Bass program (cached per process).

    repeat>1 runs the whole per-core conv `repeat` times back-to-back inside
    one NEFF (idempotent) — used by test.py to measure steady-state device
    time net of dispatch overhead.
    """
    key = ("nc", repeat)
    if key in _cache:
        return _cache[key]

    nc = bacc.Bacc("TRN2", target_bir_lowering=False, debug=False)
    x_ap = nc.dram_tensor(
        "x", [BPC, C, HP, WP], MM_DTYPE, kind="ExternalInput"
    ).ap()
    w_ap = nc.dram_tensor(
        "w", [2 * C, 6 * OC], MM_DTYPE, kind="ExternalInput"
    ).ap()
    out_ap = nc.dram_tensor(
        "out", [BPC, OC, H, WP], mybir.dt.float32, kind="ExternalOutput"
    ).ap()

    with tile.TileContext(nc) as tc:
        with (
            tc.tile_pool(name="xpool", bufs=2) as xpool,
            tc.tile_pool(name="wpool", bufs=1) as wpool,
            tc.tile_pool(name="opool", bufs=6) as opool,
            tc.tile_pool(name="psum", bufs=6, space="PSUM") as pspool,
        ):
            # Weight slots: 3 tap-pair slots (K=128: rows 0-63 = tap (0,d),
            # rows 64-127 = tap (1,d)) + 3 single slots (K=64: tap (2,d)).
            wt = wpool.tile([2 * C, 6 * OC], MM_DTYPE)
            nc.sync.dma_start(wt[:], w_ap[:])

            def conv_pass():
                for im in range(BPC):
                    # Partitions 0-63: padded image (rows 0-113).
                    # Partitions 64-127: same image shifted one row (+WP), so
                    # a K=128 matmul at offset j0+d contracts taps (0,d) and
                    # (1,d) simultaneously.
                    xt = xpool.tile([2 * C, XFLAT + 4], MM_DTYPE)
                    x_im = x_ap[im].rearrange("c h w -> c (h w)")
                    h = C // 2
                    nc.sync.dma_start(xt[:h, :XFLAT], x_im[:h])
                    nc.sync.dma_start(xt[h : 2 * h, :XFLAT], x_im[h:])
                    nb = XFLAT - WP
                    nc.sync.dma_start(xt[2 * h : 3 * h, :nb], x_im[:h, WP:])
                    nc.sync.dma_start(xt[3 * h :, :nb], x_im[h:, WP:])
                    o_im = out_ap[im].rearrange("o h w -> o (h w)")

                    for blk in range(NBLK):
                        j0 = blk * BLK
                        n = min(BLK, OFLAT - j0)
                        ps = pspool.tile([OC, BLK], mybir.dt.float32)
                        for d in range(3):
                            nc.tensor.matmul(
                                ps[:, :n],
                                lhsT=wt[:, d * OC : (d + 1) * OC],
                                rhs=xt[:, j0 + d : j0 + d + n],
                                start=(d == 0),
                                stop=False,
                            )
                        for d in range(3):
                            nc.tensor.matmul(
                                ps[:, :n],
                                lhsT=wt[:C, (3 + d) * OC : (4 + d) * OC],
                                rhs=xt[:C, j0 + 2 * WP + d : j0 + 2 * WP + d + n],
                                start=False,
                                stop=(d == 2),
                            )
                        ot = opool.tile([OC, BLK], mybir.dt.float32)
                        nc.vector.tensor_copy(ot[:, :n], ps[:, :n])
                        nc.sync.dma_start(o_im[:, j0 : j0 + n], ot[:, :n])

            if repeat == 1:
                conv_pass()
            else:
                with tc.For_i(0, repeat, 1):
                    conv_pass()

    nc.compile()
    _cache[key] = nc
    return nc


def run_on_device(nc, in_maps):
    """Single-exec jitted runner with device-resident inputs; returns a
    callable for repeated timing plus the output fetcher."""
    from jax.sharding import Mesh, NamedSharding, PartitionSpec
    from jax.experimental.shard_map import shard_map
    import jax

    from concourse.bass2jax import (
        _bass_exec_p,
        install_neuronx_cc_hook,
        partition_id_tensor,
    )

    install_neuronx_cc_hook()

    partition_name = nc.partition_id_tensor.name if nc.partition_id_tensor else None
    in_names, out_names, out_avals = [], [], []
    for alloc in nc.m.functions[0].allocations:
        if not isinstance(alloc, mybir.MemoryLocationSet):
            continue
        name = alloc.memorylocations[0].name
        if alloc.kind == "ExternalInput":
            if name != partition_name:
                in_names.append(name)
        elif alloc.kind == "ExternalOutput":
            out_names.append(name)
            out_avals.append(
                jax.core.ShapedArray(
                    tuple(alloc.tensor_shape), mybir.dt.np(alloc.dtype)
                )
            )
    n_params = len(in_names)
    all_in_names = list(in_names) + list(out_names)
    if partition_name is not None:
        all_in_names.append(partition_name)
    all_in_names = tuple(all_in_names)

    def body(*args):
        operands = list(args)
        if partition_name is not None:
            operands.append(partition_id_tensor())
        return tuple(
            _bass_exec_p.bind(
                *operands,
                out_avals=tuple(out_avals),
                in_names=all_in_names,
                out_names=tuple(out_names),
                lowering_input_output_aliases=(),
                sim_require_finite=True,
                sim_require_nnan=True,
                nc=nc,
            )
        )

    n_cores = len(in_maps)
    devices = jax.devices()[:n_cores]
    mesh = Mesh(np.asarray(devices), ("core",))
    nspecs = n_params + len(out_names)
    sharded = jax.jit(
        shard_map(
            body,
            mesh=mesh,
            in_specs=(PartitionSpec("core"),) * nspecs,
            out_specs=(PartitionSpec("core"),) * len(out_names),
            check_rep=False,
        )
    )
    concat_in = [
        np.concatenate([np.asarray(in_maps[c][nm]) for c in range(n_cores)], axis=0)
        for nm in in_names
    ]
    concat_zeros = [
        np.zeros((n_cores * a.shape[0], *a.shape[1:]), a.dtype) for a in out_avals
    ]
    sharding = NamedSharding(mesh, PartitionSpec("core"))
    dev_in = [jax.device_put(a, sharding) for a in concat_in]
    dev_zeros = [jax.device_put(a, sharding) for a in concat_zeros]

    def run():
        return sharded(*dev_in, *dev_zeros)

    return run, out_names, out_avals


def _prep_inputs(x, weight):
    """Host-side shard + layout prep. Returns per-core input maps."""
    xp = np.zeros((B, C, HP, WP), dtype=np.float32)
    xp[:, :, 1 : H + 1, 1 : W + 1] = x
    # lhsT slots: pairs d=0..2 pack taps (0,d) [rows 0-63] + (1,d)
    # [rows 64-127]; singles 3+d hold tap (2,d) in rows 0-63.
    w4 = weight[0]  # (out_c, in_c, kh, kw)
    wp = np.zeros((2 * C, 6, OC), dtype=np.float32)
    for d in range(KW):
        wp[:C, d] = w4[:, :, 0, d].T
        wp[C:, d] = w4[:, :, 1, d].T
        wp[:C, 3 + d] = w4[:, :, 2, d].T
    w_prep = np.ascontiguousarray(wp.reshape(2 * C, 6 * OC))
    return [
        {"x": xp[c * BPC : (c + 1) * BPC], "w": w_prep} for c in range(NCORES)
    ]


def kernel(x, weight):
    x = np.asarray(x, dtype=np.float32)
    weight = np.asarray(weight, dtype=np.float32)
    nc = _build()
    in_maps = _prep_inputs(x, weight)
    res = run_bass_kernel_spmd(nc, in_maps, list(range(NCORES)))
    out = np.concatenate([res.results[c]["out"] for c in range(NCORES)], axis=0)
    return np.ascontiguousarray(out[:, :, :, :W])


# revision 48
# speedup vs baseline: 2.5611x; 2.5611x over previous
"""Trainium2 Bass kernel: 3x3 conv2d (stride 1, pad 1), NCHW.

x (32, 64, 112, 112) f32, weight (1, 128, 64, 3, 3) f32 -> out (32, 128, 112, 112) f32.

Strategy: data-parallel over batch across 8 cores (4 images/core).
Per core, conv is computed as 9 PSUM-accumulating matmuls (one per kernel
tap): x is host-padded to (114, 114) so each tap's shifted input window is a
constant free-dim offset into the flat [in_c=64, 114*114] SBUF image. Output
is produced in padded row-major (112 x 114) layout and sliced on the host.
"""

import numpy as np

import concourse.bacc as bacc
import concourse.tile as tile
from concourse import mybir
from concourse.bass_utils import run_bass_kernel_spmd

# Problem constants (hardcoded per harness contract).
B, C, H, W = 32, 64, 112, 112
OC, KH, KW = 128, 3, 3
NCORES = 8
BPC = B // NCORES          # images per core
HP, WP = H + 2, W + 2      # host-padded input height/width (114)
XFLAT = HP * WP            # 12996 flat padded-input elements per channel
OFLAT = H * WP             # 12768 flat padded-output elements per channel
BLK = 512                  # matmul free-dim block (= 1 PSUM bank of fp32)
NBLK = (OFLAT + BLK - 1) // BLK  # 25 blocks (24 full + 1 of 480)
XBUF = XFLAT + 4           # SBUF image stride (matmul offsets read to XFLAT+1)
GS = 4                     # out-DMA grouping: 4 blocks -> 1 MiB transfers

# matmul dtype: float32r streams fp32 through the PE at 1 cycle/row for
# free-dim >= 256 (vs 4 cycles/row for plain float32).
MM_DTYPE = mybir.dt.float32r

_cache = {}

# Variant switch (test harness flips this to isolate bottlenecks):
#   "pack6k128_fp16" (default) - fp16 operands, 6 all-K=128 MMs per block:
#        3 tap-pair MMs (taps (0,d)+(1,d) via the one-row-shifted copy on
#        partitions 64-127) + 3 "half-pair" MMs ([0; w(2,d)] at offset WP+d).
#        Measured ~2.9e-4 rel err; K=64 matmuls hit a much slower walrus
#        path, hence all-K=128.
#   "pack6"        - fp32r: 3 K=128 tap-pair MMs + 3 K=64 single MMs per block
#   "pack6_bf16"   - same structure as pack6, bf16 operands
#   "pack6k128_bf16" - as default but bf16
#   "wsplit9_bf16" - bf16 x on both partition halves; lhsT packs [w_hi; w_lo]
#                    per tap (w ~ w_hi + w_lo, near-fp32 weight precision);
#                    9 K=128 MMs per block
#   "wsplit9_ldw"  - wsplit9 tap-major with ldweights=False reuse (slower)
#   "mm_only" / "dma_only" - bottleneck-isolation probes
VARIANT = "pack6k128_fp16"


def _build(repeat=1):
    """Build + compile the per-core Bass program (cached per process).

    repeat>1 runs the whole per-core conv `repeat` times back-to-back inside
    one NEFF (idempotent) — used by test.py to measure steady-state device
    time net of dispatch overhead.
    """
    key = ("nc", repeat, VARIANT)
    if key in _cache:
        return _cache[key]
    variant = VARIANT

    nc = bacc.Bacc("TRN2", target_bir_lowering=False, debug=False)
    if variant.endswith("fp16"):
        mm_dt = mybir.dt.float16
    elif variant.endswith("bf16") or variant in ("mm_only", "wsplit9_ldw"):
        mm_dt = mybir.dt.bfloat16
    else:
        mm_dt = MM_DTYPE
    nslot = 9 if variant in ("wsplit9_bf16", "mm_only", "wsplit9_ldw") else 6
    assert variant in (
        "pack6",
        "pack6_bf16",
        "pack6k128_bf16",
        "pack6k128_fp16",
        "wsplit9_bf16",
        "wsplit9_ldw",
        "mm_only",
        "dma_only",
    ), variant
    # x arrives pre-doubled from the host: per image a [128, XBUF] block whose
    # partitions 0-63 hold the padded image (rows 0-113) and partitions 64-127
    # the same image shifted one row (pack6*) or repeated (wsplit9), so one
    # full-width DMA loads both copies.
    x_ap = nc.dram_tensor(
        "x", [BPC, 2 * C, XBUF], mm_dt, kind="ExternalInput"
    ).ap()
    w_ap = nc.dram_tensor(
        "w", [2 * C, nslot * OC], mm_dt, kind="ExternalInput"
    ).ap()
    out_ap = nc.dram_tensor(
        "out", [BPC, OC, H, WP], mybir.dt.float32, kind="ExternalOutput"
    ).ap()

    with tile.TileContext(nc) as tc:
        with (
            tc.tile_pool(name="xpool", bufs=2) as xpool,
            tc.tile_pool(name="wpool", bufs=1) as wpool,
            tc.tile_pool(name="opool", bufs=4) as opool,
            tc.tile_pool(name="psum", bufs=8, space="PSUM") as pspool,
        ):
            # Weight slots: pack6* = 3 tap-pair slots (K=128: rows 0-63 =
            # tap (0,d), rows 64-127 = tap (1,d)) + 3 single slots (K=64:
            # tap (2,d)); wsplit9 = 9 taps x [w_hi; w_lo].
            wt = wpool.tile([2 * C, nslot * OC], mm_dt)
            nc.sync.dma_start(wt[:], w_ap[:])

            def conv_pass():
                for im in range(BPC):
                    # Partitions 0-63: padded image (rows 0-113).
                    # Partitions 64-127: same image shifted one row (+WP), so
                    # a K=128 matmul at offset j0+d contracts taps (0,d) and
                    # (1,d) simultaneously.
                    xt = xpool.tile([2 * C, XBUF], mm_dt)
                    nc.sync.dma_start(xt[:], x_ap[im])
                    o_im = out_ap[im].rearrange("o h w -> o (h w)")

                    if variant == "wsplit9_ldw":
                        # Tap-major over groups of GS blocks: one weight load
                        # per tap per group; the other GS-1 matmuls reuse the
                        # loaded weights (ldweights=False). PE instructions
                        # execute in FIFO program order, so the pairing holds.
                        for g0 in range(0, OFLAT, GS * BLK):
                            blks = [
                                (j0, min(BLK, OFLAT - j0))
                                for j0 in range(g0, min(g0 + GS * BLK, OFLAT), BLK)
                            ]
                            pss = [
                                pspool.tile(
                                    [OC, BLK],
                                    mybir.dt.float32,
                                    name=f"ps{bi}",
                                    tag="ps",
                                )
                                for bi in range(len(blks))
                            ]
                            for t in range(KH * KW):
                                dh, dw = divmod(t, KW)
                                for bi, (j0, n) in enumerate(blks):
                                    off = j0 + dh * WP + dw
                                    mm = nc.tensor.matmul(
                                        pss[bi][:, :n],
                                        lhsT=wt[:, t * OC : (t + 1) * OC],
                                        rhs=xt[:, off : off + n],
                                        start=(t == 0),
                                        stop=(t == KH * KW - 1),
                                    )
                                    if bi > 0:
                                        mm.ldweights = False
                            ot = opool.tile([OC, GS * BLK], mybir.dt.float32)
                            for bi, (j0, n) in enumerate(blks):
                                nc.vector.tensor_copy(
                                    ot[:, bi * BLK : bi * BLK + n], pss[bi][:, :n]
                                )
                            gn = blks[-1][0] + blks[-1][1] - g0
                            nc.sync.dma_start(o_im[:, g0 : g0 + gn], ot[:, :gn])
                        continue

                    ot = None
                    for blk in range(NBLK):
                        j0 = blk * BLK
                        n = min(BLK, OFLAT - j0)
                        g = blk % GS
                        if g == 0:
                            ot = opool.tile([OC, GS * BLK], mybir.dt.float32)
                            g0 = j0
                        if variant == "dma_only":
                            nc.vector.tensor_copy(
                                ot[:, g * BLK : g * BLK + n], xt[:OC, j0 : j0 + n]
                            )
                        elif variant in ("wsplit9_bf16", "mm_only"):
                            ps = pspool.tile([OC, BLK], mybir.dt.float32)
                            for t in range(KH * KW):
                                dh, dw = divmod(t, KW)
                                off = j0 + dh * WP + dw
                                nc.tensor.matmul(
                                    ps[:, :n],
                                    lhsT=wt[:, t * OC : (t + 1) * OC],
                                    rhs=xt[:, off : off + n],
                                    start=(t == 0),
                                    stop=(t == KH * KW - 1),
                                )
                            if variant == "mm_only":
                                continue
                            nc.vector.tensor_copy(
                                ot[:, g * BLK : g * BLK + n], ps[:, :n]
                            )
                        else:
                            ps = pspool.tile([OC, BLK], mybir.dt.float32)
                            k128 = variant.startswith("pack6k128")
                            for d in range(3):
                                nc.tensor.matmul(
                                    ps[:, :n],
                                    lhsT=wt[:, d * OC : (d + 1) * OC],
                                    rhs=xt[:, j0 + d : j0 + d + n],
                                    start=(d == 0),
                                    stop=False,
                                )
                            for d in range(3):
                                if k128:
                                    # Slot 3+d = [0; w(2,d)]: upper half (copy
                                    # B, +WP shift) contributes tap (2,d) at
                                    # offset WP+d; lower half is zeroed.
                                    nc.tensor.matmul(
                                        ps[:, :n],
                                        lhsT=wt[:, (3 + d) * OC : (4 + d) * OC],
                                        rhs=xt[:, j0 + WP + d : j0 + WP + d + n],
                                        start=False,
                                        stop=(d == 2),
                                    )
                                else:
                                    nc.tensor.matmul(
                                        ps[:, :n],
                                        lhsT=wt[:C, (3 + d) * OC : (4 + d) * OC],
                                        rhs=xt[
                                            :C,
                                            j0 + 2 * WP + d : j0 + 2 * WP + d + n,
                                        ],
                                        start=False,
                                        stop=(d == 2),
                                    )
                            nc.vector.tensor_copy(
                                ot[:, g * BLK : g * BLK + n], ps[:, :n]
                            )
                        if g == GS - 1 or blk == NBLK - 1:
                            gn = j0 + n - g0
                            nc.sync.dma_start(
                                o_im[:, g0 : g0 + gn], ot[:, :gn]
                            )

            if repeat == 1:
                conv_pass()
            else:
                with tc.For_i(0, repeat, 1):
                    conv_pass()

    nc.compile()
    _cache[key] = nc
    return nc


def run_on_device(nc, in_maps):
    """Single-exec jitted runner with device-resident inputs; returns a
    callable for repeated timing plus the output fetcher."""
    from jax.sharding import Mesh, NamedSharding, PartitionSpec
    from jax.experimental.shard_map import shard_map
    import jax

    from concourse.bass2jax import (
        _bass_exec_p,
        install_neuronx_cc_hook,
        partition_id_tensor,
    )

    install_neuronx_cc_hook()

    partition_name = nc.partition_id_tensor.name if nc.partition_id_tensor else None
    in_names, out_names, out_avals = [], [], []
    for alloc in nc.m.functions[0].allocations:
        if not isinstance(alloc, mybir.MemoryLocationSet):
            continue
        name = alloc.memorylocations[0].name
        if alloc.kind == "ExternalInput":
            if name != partition_name:
                in_names.append(name)
        elif alloc.kind == "ExternalOutput":
            out_names.append(name)
            out_avals.append(
                jax.core.ShapedArray(
                    tuple(alloc.tensor_shape), mybir.dt.np(alloc.dtype)
                )
            )
    n_params = len(in_names)
    all_in_names = list(in_names) + list(out_names)
    if partition_name is not None:
        all_in_names.append(partition_name)
    all_in_names = tuple(all_in_names)

    def body(*args):
        operands = list(args)
        if partition_name is not None:
            operands.append(partition_id_tensor())
        return tuple(
            _bass_exec_p.bind(
                *operands,
                out_avals=tuple(out_avals),
                in_names=all_in_names,
                out_names=tuple(out_names),
                lowering_input_output_aliases=(),
                sim_require_finite=True,
                sim_require_nnan=True,
                nc=nc,
            )
        )

    n_cores = len(in_maps)
    devices = jax.devices()[:n_cores]
    mesh = Mesh(np.asarray(devices), ("core",))
    nspecs = n_params + len(out_names)
    sharded = jax.jit(
        shard_map(
            body,
            mesh=mesh,
            in_specs=(PartitionSpec("core"),) * nspecs,
            out_specs=(PartitionSpec("core"),) * len(out_names),
            check_rep=False,
        )
    )
    concat_in = [
        np.concatenate([np.asarray(in_maps[c][nm]) for c in range(n_cores)], axis=0)
        for nm in in_names
    ]
    concat_zeros = [
        np.zeros((n_cores * a.shape[0], *a.shape[1:]), a.dtype) for a in out_avals
    ]
    sharding = NamedSharding(mesh, PartitionSpec("core"))
    dev_in = [jax.device_put(a, sharding) for a in concat_in]
    dev_zeros = [jax.device_put(a, sharding) for a in concat_zeros]

    def run():
        return sharded(*dev_in, *dev_zeros)

    return run, out_names, out_avals


def _prep_inputs(x, weight):
    """Host-side shard + layout prep. Returns per-core input maps."""
    import ml_dtypes

    variant = VARIANT
    if variant.endswith("fp16"):
        host_dt = np.float16
    elif variant.endswith("bf16") or variant in ("mm_only", "wsplit9_ldw"):
        host_dt = ml_dtypes.bfloat16
    else:
        host_dt = np.float32

    xp = np.zeros((B, C, HP, WP), dtype=np.float32)
    xp[:, :, 1 : H + 1, 1 : W + 1] = x
    flat = xp.reshape(B, C, XFLAT).astype(host_dt)
    xprep = np.zeros((B, 2 * C, XBUF), dtype=host_dt)
    xprep[:, :C, :XFLAT] = flat
    if variant in ("wsplit9_bf16", "mm_only", "wsplit9_ldw"):
        xprep[:, C:, :XFLAT] = flat
    else:
        xprep[:, C:, : XFLAT - WP] = flat[:, :, WP:]

    w4 = weight[0]  # (out_c, in_c, kh, kw)
    if variant in ("wsplit9_bf16", "mm_only", "wsplit9_ldw"):
        # lhsT slot t: rows 0-63 = bf16(w[tap t]), rows 64-127 = bf16 of the
        # residual -> contraction over both halves gives ~fp32 weight
        # precision at bf16 matmul rate.
        w_hi = w4.astype(ml_dtypes.bfloat16)
        w_lo = (w4.astype(np.float32) - w_hi.astype(np.float32)).astype(
            ml_dtypes.bfloat16
        )
        wp = np.zeros((2 * C, KH * KW, OC), dtype=host_dt)
        for t in range(KH * KW):
            kh, kw = divmod(t, KW)
            wp[:C, t] = w_hi[:, :, kh, kw].T
            wp[C:, t] = w_lo[:, :, kh, kw].T
        w_prep = np.ascontiguousarray(wp.reshape(2 * C, KH * KW * OC))
    else:
        # lhsT slots: pairs d=0..2 pack taps (0,d) [rows 0-63] + (1,d)
        # [rows 64-127]. Singles 3+d hold tap (2,d): in rows 0-63 for the
        # K=64 variant, in rows 64-127 (zero top, used with the +WP-shifted
        # copy at offset WP+d) for the all-K=128 variant.
        wp = np.zeros((2 * C, 6, OC), dtype=host_dt)
        for d in range(KW):
            wp[:C, d] = w4[:, :, 0, d].T.astype(host_dt)
            wp[C:, d] = w4[:, :, 1, d].T.astype(host_dt)
            if variant.startswith("pack6k128"):
                wp[C:, 3 + d] = w4[:, :, 2, d].T.astype(host_dt)
            else:
                wp[:C, 3 + d] = w4[:, :, 2, d].T.astype(host_dt)
        w_prep = np.ascontiguousarray(wp.reshape(2 * C, 6 * OC))
    return [
        {"x": xprep[c * BPC : (c + 1) * BPC], "w": w_prep} for c in range(NCORES)
    ]


def kernel(x, weight):
    x = np.asarray(x, dtype=np.float32)
    weight = np.asarray(weight, dtype=np.float32)
    nc = _build()
    in_maps = _prep_inputs(x, weight)
    res = run_bass_kernel_spmd(nc, in_maps, list(range(NCORES)))
    out = np.concatenate([res.results[c]["out"] for c in range(NCORES)], axis=0)
    return np.ascontiguousarray(out[:, :, :, :W])


# revision 53
# speedup vs baseline: 2.5750x; 1.0054x over previous
"""Trainium2 Bass kernel: 3x3 conv2d (stride 1, pad 1), NCHW.

x (32, 64, 112, 112) f32, weight (1, 128, 64, 3, 3) f32 -> out (32, 128, 112, 112) f32.

Strategy: data-parallel over batch across 8 cores (4 images/core).
Per core, conv is computed as 9 PSUM-accumulating matmuls (one per kernel
tap): x is host-padded to (114, 114) so each tap's shifted input window is a
constant free-dim offset into the flat [in_c=64, 114*114] SBUF image. Output
is produced in padded row-major (112 x 114) layout and sliced on the host.
"""

import numpy as np

import concourse.bacc as bacc
import concourse.tile as tile
from concourse import mybir
from concourse.bass_utils import run_bass_kernel_spmd

# Problem constants (hardcoded per harness contract).
B, C, H, W = 32, 64, 112, 112
OC, KH, KW = 128, 3, 3
NCORES = 8
BPC = B // NCORES          # images per core
HP, WP = H + 2, W + 2      # host-padded input height/width (114)
XFLAT = HP * WP            # 12996 flat padded-input elements per channel
OFLAT = H * WP             # 12768 flat padded-output elements per channel
BLK = 512                  # matmul free-dim block (= 1 PSUM bank of fp32)
NBLK = (OFLAT + BLK - 1) // BLK  # 25 blocks (24 full + 1 of 480)
XBUF = XFLAT + 4           # SBUF image stride (matmul offsets read to XFLAT+1)
GS = 8                     # out-DMA grouping: 8 blocks per transfer
# Emit the output in fp16 (DVE casts during the PSUM->SBUF drain; the host
# upcasts to fp32). Halves the dominant out-DMA traffic for ~2^-11 extra
# rounding error. Applies to fp16 variants only.
OUT_FP16 = True

# matmul dtype: float32r streams fp32 through the PE at 1 cycle/row for
# free-dim >= 256 (vs 4 cycles/row for plain float32).
MM_DTYPE = mybir.dt.float32r

_cache = {}

# Variant switch (test harness flips this to isolate bottlenecks):
#   "pack6k128_fp16" (default) - fp16 operands, 6 all-K=128 MMs per block:
#        3 tap-pair MMs (taps (0,d)+(1,d) via the one-row-shifted copy on
#        partitions 64-127) + 3 "half-pair" MMs ([0; w(2,d)] at offset WP+d).
#        Measured ~2.9e-4 rel err; K=64 matmuls hit a much slower walrus
#        path, hence all-K=128.
#   "pack6"        - fp32r: 3 K=128 tap-pair MMs + 3 K=64 single MMs per block
#   "pack6_bf16"   - same structure as pack6, bf16 operands
#   "pack6k128_bf16" - as default but bf16
#   "wsplit9_bf16" - bf16 x on both partition halves; lhsT packs [w_hi; w_lo]
#                    per tap (w ~ w_hi + w_lo, near-fp32 weight precision);
#                    9 K=128 MMs per block
#   "wsplit9_ldw"  - wsplit9 tap-major with ldweights=False reuse (slower)
#   "mm_only" / "dma_only" - bottleneck-isolation probes
VARIANT = "pack6k128_fp16"


def _build(repeat=1):
    """Build + compile the per-core Bass program (cached per process).

    repeat>1 runs the whole per-core conv `repeat` times back-to-back inside
    one NEFF (idempotent) — used by test.py to measure steady-state device
    time net of dispatch overhead.
    """
    key = ("nc", repeat, VARIANT)
    if key in _cache:
        return _cache[key]
    variant = VARIANT

    nc = bacc.Bacc("TRN2", target_bir_lowering=False, debug=False)
    if variant.endswith("fp16"):
        mm_dt = mybir.dt.float16
    elif variant.endswith("bf16") or variant in ("mm_only", "wsplit9_ldw"):
        mm_dt = mybir.dt.bfloat16
    else:
        mm_dt = MM_DTYPE
    nslot = 9 if variant in ("wsplit9_bf16", "mm_only", "wsplit9_ldw") else 6
    assert variant in (
        "pack6",
        "pack6_bf16",
        "pack6k128_bf16",
        "pack6k128_fp16",
        "wsplit9_bf16",
        "wsplit9_ldw",
        "mm_only",
        "dma_only",
    ), variant
    # x arrives pre-doubled from the host: per image a [128, XBUF] block whose
    # partitions 0-63 hold the padded image (rows 0-113) and partitions 64-127
    # the same image shifted one row (pack6*) or repeated (wsplit9), so one
    # full-width DMA loads both copies.
    x_ap = nc.dram_tensor(
        "x", [BPC, 2 * C, XBUF], mm_dt, kind="ExternalInput"
    ).ap()
    w_ap = nc.dram_tensor(
        "w", [2 * C, nslot * OC], mm_dt, kind="ExternalInput"
    ).ap()
    out_dt = (
        mybir.dt.float16
        if (OUT_FP16 and mm_dt == mybir.dt.float16)
        else mybir.dt.float32
    )
    out_ap = nc.dram_tensor(
        "out", [BPC, OC, H, WP], out_dt, kind="ExternalOutput"
    ).ap()

    with tile.TileContext(nc) as tc:
        with (
            tc.tile_pool(name="xpool", bufs=3) as xpool,
            tc.tile_pool(name="wpool", bufs=1) as wpool,
            tc.tile_pool(name="opool", bufs=4) as opool,
            tc.tile_pool(name="psum", bufs=8, space="PSUM") as pspool,
        ):
            # Weight slots: pack6* = 3 tap-pair slots (K=128: rows 0-63 =
            # tap (0,d), rows 64-127 = tap (1,d)) + 3 single slots (K=64:
            # tap (2,d)); wsplit9 = 9 taps x [w_hi; w_lo].
            wt = wpool.tile([2 * C, nslot * OC], mm_dt)
            nc.sync.dma_start(wt[:], w_ap[:])

            def conv_pass():
                for im in range(BPC):
                    # Partitions 0-63: padded image (rows 0-113).
                    # Partitions 64-127: same image shifted one row (+WP), so
                    # a K=128 matmul at offset j0+d contracts taps (0,d) and
                    # (1,d) simultaneously.
                    xt = xpool.tile([2 * C, XBUF], mm_dt)
                    nc.sync.dma_start(xt[:], x_ap[im])
                    o_im = out_ap[im].rearrange("o h w -> o (h w)")

                    if variant == "wsplit9_ldw":
                        # Tap-major over groups of GS blocks: one weight load
                        # per tap per group; the other GS-1 matmuls reuse the
                        # loaded weights (ldweights=False). PE instructions
                        # execute in FIFO program order, so the pairing holds.
                        for g0 in range(0, OFLAT, GS * BLK):
                            blks = [
                                (j0, min(BLK, OFLAT - j0))
                                for j0 in range(g0, min(g0 + GS * BLK, OFLAT), BLK)
                            ]
                            pss = [
                                pspool.tile(
                                    [OC, BLK],
                                    mybir.dt.float32,
                                    name=f"ps{bi}",
                                    tag="ps",
                                )
                                for bi in range(len(blks))
                            ]
                            for t in range(KH * KW):
                                dh, dw = divmod(t, KW)
                                for bi, (j0, n) in enumerate(blks):
                                    off = j0 + dh * WP + dw
                                    mm = nc.tensor.matmul(
                                        pss[bi][:, :n],
                                        lhsT=wt[:, t * OC : (t + 1) * OC],
                                        rhs=xt[:, off : off + n],
                                        start=(t == 0),
                                        stop=(t == KH * KW - 1),
                                    )
                                    if bi > 0:
                                        mm.ldweights = False
                            ot = opool.tile([OC, GS * BLK], out_dt)
                            for bi, (j0, n) in enumerate(blks):
                                nc.vector.tensor_copy(
                                    ot[:, bi * BLK : bi * BLK + n], pss[bi][:, :n]
                                )
                            gn = blks[-1][0] + blks[-1][1] - g0
                            nc.sync.dma_start(o_im[:, g0 : g0 + gn], ot[:, :gn])
                        continue

                    ot = None
                    for blk in range(NBLK):
                        j0 = blk * BLK
                        n = min(BLK, OFLAT - j0)
                        g = blk % GS
                        if g == 0:
                            ot = opool.tile([OC, GS * BLK], out_dt)
                            g0 = j0
                        if variant == "dma_only":
                            nc.vector.tensor_copy(
                                ot[:, g * BLK : g * BLK + n], xt[:OC, j0 : j0 + n]
                            )
                        elif variant in ("wsplit9_bf16", "mm_only"):
                            ps = pspool.tile([OC, BLK], mybir.dt.float32)
                            for t in range(KH * KW):
                                dh, dw = divmod(t, KW)
                                off = j0 + dh * WP + dw
                                nc.tensor.matmul(
                                    ps[:, :n],
                                    lhsT=wt[:, t * OC : (t + 1) * OC],
                                    rhs=xt[:, off : off + n],
                                    start=(t == 0),
                                    stop=(t == KH * KW - 1),
                                )
                            if variant == "mm_only":
                                continue
                            nc.vector.tensor_copy(
                                ot[:, g * BLK : g * BLK + n], ps[:, :n]
                            )
                        else:
                            ps = pspool.tile([OC, BLK], mybir.dt.float32)
                            k128 = variant.startswith("pack6k128")
                            for d in range(3):
                                nc.tensor.matmul(
                                    ps[:, :n],
                                    lhsT=wt[:, d * OC : (d + 1) * OC],
                                    rhs=xt[:, j0 + d : j0 + d + n],
                                    start=(d == 0),
                                    stop=False,
                                )
                            for d in range(3):
                                if k128:
                                    # Slot 3+d = [0; w(2,d)]: upper half (copy
                                    # B, +WP shift) contributes tap (2,d) at
                                    # offset WP+d; lower half is zeroed.
                                    nc.tensor.matmul(
                                        ps[:, :n],
                                        lhsT=wt[:, (3 + d) * OC : (4 + d) * OC],
                                        rhs=xt[:, j0 + WP + d : j0 + WP + d + n],
                                        start=False,
                                        stop=(d == 2),
                                    )
                                else:
                                    nc.tensor.matmul(
                                        ps[:, :n],
                                        lhsT=wt[:C, (3 + d) * OC : (4 + d) * OC],
                                        rhs=xt[
                                            :C,
                                            j0 + 2 * WP + d : j0 + 2 * WP + d + n,
                                        ],
                                        start=False,
                                        stop=(d == 2),
                                    )
                            nc.vector.tensor_copy(
                                ot[:, g * BLK : g * BLK + n], ps[:, :n]
                            )
                        if g == GS - 1 or blk == NBLK - 1:
                            gn = j0 + n - g0
                            nc.sync.dma_start(
                                o_im[:, g0 : g0 + gn], ot[:, :gn]
                            )

            if repeat == 1:
                conv_pass()
            else:
                with tc.For_i(0, repeat, 1):
                    conv_pass()

    nc.compile()
    _cache[key] = nc
    return nc


def run_on_device(nc, in_maps):
    """Single-exec jitted runner with device-resident inputs; returns a
    callable for repeated timing plus the output fetcher."""
    from jax.sharding import Mesh, NamedSharding, PartitionSpec
    from jax.experimental.shard_map import shard_map
    import jax

    from concourse.bass2jax import (
        _bass_exec_p,
        install_neuronx_cc_hook,
        partition_id_tensor,
    )

    install_neuronx_cc_hook()

    partition_name = nc.partition_id_tensor.name if nc.partition_id_tensor else None
    in_names, out_names, out_avals = [], [], []
    for alloc in nc.m.functions[0].allocations:
        if not isinstance(alloc, mybir.MemoryLocationSet):
            continue
        name = alloc.memorylocations[0].name
        if alloc.kind == "ExternalInput":
            if name != partition_name:
                in_names.append(name)
        elif alloc.kind == "ExternalOutput":
            out_names.append(name)
            out_avals.append(
                jax.core.ShapedArray(
                    tuple(alloc.tensor_shape), mybir.dt.np(alloc.dtype)
                )
            )
    n_params = len(in_names)
    all_in_names = list(in_names) + list(out_names)
    if partition_name is not None:
        all_in_names.append(partition_name)
    all_in_names = tuple(all_in_names)

    def body(*args):
        operands = list(args)
        if partition_name is not None:
            operands.append(partition_id_tensor())
        return tuple(
            _bass_exec_p.bind(
                *operands,
                out_avals=tuple(out_avals),
                in_names=all_in_names,
                out_names=tuple(out_names),
                lowering_input_output_aliases=(),
                sim_require_finite=True,
                sim_require_nnan=True,
                nc=nc,
            )
        )

    n_cores = len(in_maps)
    devices = jax.devices()[:n_cores]
    mesh = Mesh(np.asarray(devices), ("core",))
    nspecs = n_params + len(out_names)
    sharded = jax.jit(
        shard_map(
            body,
            mesh=mesh,
            in_specs=(PartitionSpec("core"),) * nspecs,
            out_specs=(PartitionSpec("core"),) * len(out_names),
            check_rep=False,
        )
    )
    concat_in = [
        np.concatenate([np.asarray(in_maps[c][nm]) for c in range(n_cores)], axis=0)
        for nm in in_names
    ]
    concat_zeros = [
        np.zeros((n_cores * a.shape[0], *a.shape[1:]), a.dtype) for a in out_avals
    ]
    sharding = NamedSharding(mesh, PartitionSpec("core"))
    dev_in = [jax.device_put(a, sharding) for a in concat_in]
    dev_zeros = [jax.device_put(a, sharding) for a in concat_zeros]

    def run():
        return sharded(*dev_in, *dev_zeros)

    return run, out_names, out_avals


def _prep_inputs(x, weight):
    """Host-side shard + layout prep. Returns per-core input maps."""
    import ml_dtypes

    variant = VARIANT
    if variant.endswith("fp16"):
        host_dt = np.float16
    elif variant.endswith("bf16") or variant in ("mm_only", "wsplit9_ldw"):
        host_dt = ml_dtypes.bfloat16
    else:
        host_dt = np.float32

    xp = np.zeros((B, C, HP, WP), dtype=np.float32)
    xp[:, :, 1 : H + 1, 1 : W + 1] = x
    flat = xp.reshape(B, C, XFLAT).astype(host_dt)
    xprep = np.zeros((B, 2 * C, XBUF), dtype=host_dt)
    xprep[:, :C, :XFLAT] = flat
    if variant in ("wsplit9_bf16", "mm_only", "wsplit9_ldw"):
        xprep[:, C:, :XFLAT] = flat
    else:
        xprep[:, C:, : XFLAT - WP] = flat[:, :, WP:]

    w4 = weight[0]  # (out_c, in_c, kh, kw)
    if variant in ("wsplit9_bf16", "mm_only", "wsplit9_ldw"):
        # lhsT slot t: rows 0-63 = bf16(w[tap t]), rows 64-127 = bf16 of the
        # residual -> contraction over both halves gives ~fp32 weight
        # precision at bf16 matmul rate.
        w_hi = w4.astype(ml_dtypes.bfloat16)
        w_lo = (w4.astype(np.float32) - w_hi.astype(np.float32)).astype(
            ml_dtypes.bfloat16
        )
        wp = np.zeros((2 * C, KH * KW, OC), dtype=host_dt)
        for t in range(KH * KW):
            kh, kw = divmod(t, KW)
            wp[:C, t] = w_hi[:, :, kh, kw].T
            wp[C:, t] = w_lo[:, :, kh, kw].T
        w_prep = np.ascontiguousarray(wp.reshape(2 * C, KH * KW * OC))
    else:
        # lhsT slots: pairs d=0..2 pack taps (0,d) [rows 0-63] + (1,d)
        # [rows 64-127]. Singles 3+d hold tap (2,d): in rows 0-63 for the
        # K=64 variant, in rows 64-127 (zero top, used with the +WP-shifted
        # copy at offset WP+d) for the all-K=128 variant.
        wp = np.zeros((2 * C, 6, OC), dtype=host_dt)
        for d in range(KW):
            wp[:C, d] = w4[:, :, 0, d].T.astype(host_dt)
            wp[C:, d] = w4[:, :, 1, d].T.astype(host_dt)
            if variant.startswith("pack6k128"):
                wp[C:, 3 + d] = w4[:, :, 2, d].T.astype(host_dt)
            else:
                wp[:C, 3 + d] = w4[:, :, 2, d].T.astype(host_dt)
        w_prep = np.ascontiguousarray(wp.reshape(2 * C, 6 * OC))
    return [
        {"x": xprep[c * BPC : (c + 1) * BPC], "w": w_prep} for c in range(NCORES)
    ]


def kernel(x, weight):
    x = np.asarray(x, dtype=np.float32)
    weight = np.asarray(weight, dtype=np.float32)
    nc = _build()
    in_maps = _prep_inputs(x, weight)
    res = run_bass_kernel_spmd(nc, in_maps, list(range(NCORES)))
    out = np.concatenate([res.results[c]["out"] for c in range(NCORES)], axis=0)
    return np.ascontiguousarray(out[:, :, :, :W].astype(np.float32))
